# revision 43
# baseline (speedup 1.0000x reference)
"""Cubic-Bezier Gaussian rasterizer for Trainium2 (Bass/Tile), 8-core SPMD.

Math (matches reference.py):
    t = linspace(0, 1, 100);  curve = Bezier3(control_points, t)   # (2, 100)
    gx[t, i] = exp(-(curve_x[t] - i/8192)^2 / 2e-4)                # (100, 8192)
    gy[t, j] = exp(-(curve_y[t] - j/8192)^2 / 2e-4)
    out = gx^T @ gy / 100                                          # (8192, 8192)

Sharding: output rows across 8 cores. Each core computes gx for its 1024
grid-row values, the full gy, and a local (1024 x 8192) matmul. No
communication; host concatenates the row slices.

Device pipeline per core (the only DMA traffic is one 2 KB input and the
32 MB output, which is the memory-regime floor):
  PE:   negc = [neg_basis; 1]^T @ [cp; rowoff] (per-core row offset folded
        into a 5th contraction row), then 128 f32r matmuls gx^T @ gy -> PSUM
  Pool: one 1024-wide iota generates the grid ramp on-chip (exact in f32);
        each chunk's column offset is folded into its Square bias
  ACT:  Square/Exp Gaussian tables (squares alternate with DVE) + ~2/5 of
        the PSUM->SBUF copies
  DVE:  the other squares + most PSUM->SBUF copies
  DMA:  column-major 512 KB stores, issued per (row-block, column) tile so
        the DMA engines saturate right after the first gy chunk

Timing (TimelineSim cost model, cross-checked on hardware by slope-fitting
wall time over an in-kernel repetition loop): ~106 us per core end-to-end;
measured steady-state pass 102.3 us including ~4 us loop overhead.
"""

import math
import os

import numpy as np

RES = 8192
STEPS = 100
N_CORES = 8
ROWS_PER_CORE = RES // N_CORES  # 1024
NEG_INV_2SIG = -5000.0  # -1 / 0.0002
LN_INV_STEPS = float(np.log(np.float64(1.0) / STEPS))

M_TILE = 128  # output rows per PE matmul (psum partition dim)
MM_N = 512  # matmul moving free dim (one PSUM bank of f32)
PS_COLS = 1024  # psum tile free size (2 banks -> 2 matmuls per copy)
GY_CHUNK = 1024  # gy chunk size for square/exp ops
N_GY = RES // GY_CHUNK  # 8

# "f32"  : exact fp32 matmul, 4 cycles/row on the PE
# "f32r" : single-pass fp32 matmul, 1 cycle/row (relaxed multiply precision)
MM_MODE = os.environ.get("BEZ_MM_MODE", "f32r")

_CACHE = {}


def _build_nc(mm_mode=None, reps=1):
    import concourse.mybir as mybir
    import concourse.tile as tile
    from concourse import bacc

    if mm_mode is None:
        mm_mode = MM_MODE
    f32 = mybir.dt.float32
    f32r = mybir.dt.float32r
    nc = bacc.Bacc(
        "TRN2", target_bir_lowering=False, debug=False, num_devices=N_CORES
    )

    # Single tiny input: [:, :100] = [neg_basis; ones] (4+1 x 100),
    # [:, 100:102] = [control_points; [row_offset, 0]] (4+1 x 2).
    comb_d = nc.dram_tensor("curve_in", [5, STEPS + 2], f32, kind="ExternalInput")
    out_d = nc.dram_tensor("out", [ROWS_PER_CORE, RES], f32, kind="ExternalOutput")

    m_tiles = ROWS_PER_CORE // M_TILE  # 8

    exp = mybir.ActivationFunctionType.Exp
    square = mybir.ActivationFunctionType.Square
    add = mybir.AluOpType.add
    mult = mybir.AluOpType.mult

    g_dt = f32r if mm_mode == "f32r" else f32

    with tile.TileContext(nc) as tc:
        with (
            tc.tile_pool(name="const", bufs=1) as const,
            tc.tile_pool(name="gyp", bufs=N_GY) as gyp,
            tc.tile_pool(name="stage", bufs=4) as stage,
            tc.tile_pool(name="obuf", bufs=6) as obuf,
            tc.tile_pool(name="psmm", bufs=3, space="PSUM") as psmm,
            tc.tile_pool(name="pscurve", bufs=1, space="PSUM") as pscurve,
        ):
            # t=0: preload the ACT Exp/Square/Copy table via a dummy op.
            lnbias = const.tile([STEPS, 1], f32)
            nc.vector.memset(lnbias, LN_INV_STEPS)
            inv_res = const.tile([STEPS, 1], f32)
            nc.vector.memset(inv_res, 1.0 / RES)
            actwarm = const.tile([STEPS, 1], f32)
            nc.scalar.activation(out=actwarm, in_=lnbias, func=exp)

            # One shared grid ramp: iota_t[t, i] = i exactly in f32. Each gy
            # chunk's column offset is folded into its per-partition Square
            # bias below, so a single 1024-wide iota serves all chunks.
            iota_t = const.tile([STEPS, GY_CHUNK], f32)
            nc.gpsimd.iota(
                iota_t,
                pattern=[[1, GY_CHUNK]],
                base=0,
                channel_multiplier=0,
                allow_small_or_imprecise_dtypes=True,
            )

            # biases[:, g] = g*GY_CHUNK/RES (memset now) + negc_y (added
            # once the curve matmul lands).
            biases = const.tile([STEPS, N_GY], f32)
            for g in range(N_GY):
                nc.vector.memset(biases[:, g : g + 1], g * GY_CHUNK / RES)

            # negc[t] = (-cx[t] + rowoff, -cy[t]): one DMA + one K=5 matmul.
            comb = const.tile([5, STEPS + 2], f32)
            nc.sync.dma_start(out=comb, in_=comb_d.ap())
            negc_ps = pscurve.tile([STEPS, 2], f32)
            nc.tensor.matmul(
                out=negc_ps,
                lhsT=comb[:, :STEPS],
                rhs=comb[:, STEPS : STEPS + 2],
                start=True,
                stop=True,
            )
            negc = const.tile([STEPS, 2], f32)
            nc.vector.tensor_copy(out=negc, in_=negc_ps)
            nc.vector.tensor_scalar(
                out=biases,
                in0=biases,
                scalar1=negc[:, 1:2],
                scalar2=None,
                op0=add,
            )

            # gx = exp(-5000*(rowoff + i/8192 - cx)^2 + ln(1/100)) [STEPS,1024]
            gxs = stage.tile([STEPS, ROWS_PER_CORE], f32, tag="gys")
            nc.scalar.activation(
                out=gxs, in_=iota_t, func=square, scale=1.0 / RES, bias=negc[:, 0:1]
            )
            gx = const.tile([STEPS, ROWS_PER_CORE], g_dt)
            nc.scalar.activation(
                out=gx, in_=gxs, func=exp, scale=NEG_INV_2SIG, bias=lnbias
            )

            gy_chunks = [None] * N_GY
            copy_state = [0]

            def emit_gy_chunk(g):
                # gy chunk g = exp(-5000*((i + g*1024)/8192 - cy)^2), read
                # from the shared ramp with the chunk offset folded into the
                # bias; squares alternate ACT/DVE to balance engines.
                gys = stage.tile([STEPS, GY_CHUNK], f32, tag="gys")
                if g % 2 == 1:
                    nc.scalar.activation(
                        out=gys,
                        in_=iota_t,
                        func=square,
                        scale=1.0 / RES,
                        bias=biases[:, g : g + 1],
                    )
                else:
                    nc.vector.tensor_scalar(
                        out=gys,
                        in0=iota_t,
                        scalar1=inv_res,
                        scalar2=biases[:, g : g + 1],
                        op0=mult,
                        op1=add,
                    )
                    nc.vector.tensor_mul(out=gys, in0=gys, in1=gys)
                gyc = gyp.tile([STEPS, GY_CHUNK], g_dt)
                nc.scalar.activation(out=gyc, in_=gys, func=exp, scale=NEG_INV_2SIG)
                gy_chunks[g] = gyc

            def emit_col_tile(mi, g):
                # one (row-block, column-chunk) tile: 2 matmuls -> PSUM,
                # one PSUM->SBUF copy, one 512 KB store.
                row0 = mi * M_TILE
                col0 = g * GY_CHUNK
                gyc = gy_chunks[g]
                ps = psmm.tile([M_TILE, PS_COLS], f32)
                for h in range(PS_COLS // MM_N):
                    nc.tensor.matmul(
                        out=ps[:, h * MM_N : (h + 1) * MM_N],
                        lhsT=gx[:, row0 : row0 + M_TILE],
                        rhs=gyc[:, h * MM_N : (h + 1) * MM_N],
                        start=True,
                        stop=True,
                    )
                ob = obuf.tile([M_TILE, GY_CHUNK], f32, tag="ob")
                # ACT also carries Gaussian-table work; it takes ~2/5 of
                # the copies, DVE the rest.
                if copy_state[0] % 5 >= 3:
                    nc.scalar.copy(out=ob, in_=ps)
                else:
                    nc.vector.tensor_copy(out=ob, in_=ps)
                copy_state[0] += 1
                nc.sync.dma_start(
                    out=out_d.ap()[row0 : row0 + M_TILE, col0 : col0 + GY_CHUNK],
                    in_=ob,
                )

            # --- main loop, column-major: as each gy chunk lands, all 8
            # row-blocks' matmuls for that column run and their 512 KB
            # tiles stream straight out. The DMA engines saturate right
            # after the first chunk and never wait on a row-block assembly.
            # (reps>1 wraps the steady state in a dynamic loop, for
            # benchmarking only.)
            if reps == 1:
                for g in range(N_GY):
                    emit_gy_chunk(g)
                    for mi in range(m_tiles):
                        emit_col_tile(mi, g)
            else:
                for g in range(N_GY):
                    emit_gy_chunk(g)
                with tc.For_i(0, reps, 1, hint_engines=(mybir.EngineType.PE,)):
                    for g in range(N_GY):
                        for mi in range(m_tiles):
                            emit_col_tile(mi, g)

    nc.compile()
    return nc


def _get_nc():
    if "nc" not in _CACHE:
        _CACHE["nc"] = _build_nc()
    return _CACHE["nc"]


def _host_constants():
    if "consts" not in _CACHE:
        t = np.linspace(0.0, 1.0, STEPS, dtype=np.float32).astype(np.float64)
        basis = np.stack(
            [math.comb(3, k) * (1.0 - t) ** (3 - k) * t**k for k in range(4)]
        )  # (4, STEPS) float64
        nb5 = np.concatenate(
            [-basis, np.ones((1, STEPS), np.float64)], axis=0
        ).astype(np.float32)  # (5, STEPS): [-basis; ones]
        _CACHE["consts"] = nb5
    return _CACHE["consts"]


TRACE = False
LAST_RESULT = None


def kernel(control_points: np.ndarray) -> np.ndarray:
    global LAST_RESULT
    from concourse.bass_utils import run_bass_kernel_spmd

    nc = _get_nc()
    nb5 = _host_constants()
    cp = np.ascontiguousarray(np.asarray(control_points), dtype=np.float32)

    in_maps = []
    for c in range(N_CORES):
        rowoff = np.float32(c * ROWS_PER_CORE) / np.float32(RES)
        cp5 = np.concatenate(
            [cp, np.array([[rowoff, 0.0]], np.float32)], axis=0
        )  # (5, 2)
        comb = np.concatenate([nb5, cp5], axis=1)  # (5, 102)
        in_maps.append({"curve_in": np.ascontiguousarray(comb)})

    res = run_bass_kernel_spmd(
        nc, in_maps, core_ids=list(range(N_CORES)), trace=TRACE
    )
    LAST_RESULT = res
    return np.concatenate([res.results[c]["out"] for c in range(N_CORES)], axis=0)


# revision 47
# speedup vs baseline: 1.0069x; 1.0069x over previous
"""Cubic-Bezier Gaussian rasterizer for Trainium2 (Bass/Tile), 8-core SPMD.

Math (matches reference.py):
    t = linspace(0, 1, 100);  curve = Bezier3(control_points, t)   # (2, 100)
    gx[t, i] = exp(-(curve_x[t] - i/8192)^2 / 2e-4)                # (100, 8192)
    gy[t, j] = exp(-(curve_y[t] - j/8192)^2 / 2e-4)
    out = gx^T @ gy / 100                                          # (8192, 8192)

Sharding: output rows across 8 cores. Each core computes gx for its 1024
grid-row values, the full gy, and a local (1024 x 8192) matmul. No
communication; host concatenates the row slices.

Device pipeline per core (the only DMA traffic is one 2 KB input and the
32 MB output, which is the memory-regime floor):
  PE:   negc = [neg_basis; 1]^T @ [cp; rowoff] (per-core row offset folded
        into a 5th contraction row), then 128 f32r matmuls gx^T @ gy -> PSUM
  Pool: one 1024-wide iota generates the grid ramp on-chip (exact in f32);
        each chunk's column offset is folded into its Square bias
  ACT:  Square/Exp Gaussian tables (squares alternate with DVE) + ~2/5 of
        the PSUM->SBUF copies
  DVE:  the other squares + most PSUM->SBUF copies
  DMA:  column-major 512 KB stores, issued per (row-block, column) tile so
        the DMA engines saturate right after the first gy chunk

Timing (TimelineSim cost model, cross-checked on hardware by slope-fitting
wall time over an in-kernel repetition loop): ~106 us per core end-to-end;
measured steady-state pass 102.3 us including ~4 us loop overhead.
"""

import math
import os

import numpy as np

RES = 8192
STEPS = 100
N_CORES = 8
ROWS_PER_CORE = RES // N_CORES  # 1024
NEG_INV_2SIG = -5000.0  # -1 / 0.0002
LN_INV_STEPS = float(np.log(np.float64(1.0) / STEPS))

M_TILE = 128  # output rows per PE matmul (psum partition dim)
MM_N = 512  # matmul moving free dim (one PSUM bank of f32)
PS_COLS = 1024  # psum tile free size (2 banks -> 2 matmuls per copy)
GY_CHUNK = 1024  # max gy chunk size for square/exp ops
# First chunks are narrow so the very first stores launch earlier; the
# rest use the full width. Must sum to RES.
GY_WIDTHS = [512, 512] + [1024] * 7
GY_OFFS = [sum(GY_WIDTHS[:i]) for i in range(len(GY_WIDTHS))]
N_GY = len(GY_WIDTHS)

# "f32"  : exact fp32 matmul, 4 cycles/row on the PE
# "f32r" : single-pass fp32 matmul, 1 cycle/row (relaxed multiply precision)
MM_MODE = os.environ.get("BEZ_MM_MODE", "f32r")

_CACHE = {}


def _build_nc(mm_mode=None, reps=1):
    import concourse.mybir as mybir
    import concourse.tile as tile
    from concourse import bacc

    if mm_mode is None:
        mm_mode = MM_MODE
    f32 = mybir.dt.float32
    f32r = mybir.dt.float32r
    nc = bacc.Bacc(
        "TRN2", target_bir_lowering=False, debug=False, num_devices=N_CORES
    )

    # Single tiny input: [:, :100] = [neg_basis; ones] (4+1 x 100),
    # [:, 100:102] = [control_points; [row_offset, 0]] (4+1 x 2).
    comb_d = nc.dram_tensor("curve_in", [5, STEPS + 2], f32, kind="ExternalInput")
    out_d = nc.dram_tensor("out", [ROWS_PER_CORE, RES], f32, kind="ExternalOutput")

    m_tiles = ROWS_PER_CORE // M_TILE  # 8

    exp = mybir.ActivationFunctionType.Exp
    square = mybir.ActivationFunctionType.Square
    add = mybir.AluOpType.add
    mult = mybir.AluOpType.mult

    g_dt = f32r if mm_mode == "f32r" else f32

    with tile.TileContext(nc) as tc:
        with (
            tc.tile_pool(name="const", bufs=1) as const,
            tc.tile_pool(name="gyp", bufs=N_GY) as gyp,
            tc.tile_pool(name="stage", bufs=4) as stage,
            tc.tile_pool(name="obuf", bufs=6) as obuf,
            tc.tile_pool(name="psmm", bufs=3, space="PSUM") as psmm,
            tc.tile_pool(name="pscurve", bufs=1, space="PSUM") as pscurve,
        ):
            # t=0: preload the ACT Exp/Square/Copy table via a dummy op.
            lnbias = const.tile([STEPS, 1], f32)
            nc.vector.memset(lnbias, LN_INV_STEPS)
            inv_res = const.tile([STEPS, 1], f32)
            nc.vector.memset(inv_res, 1.0 / RES)
            actwarm = const.tile([STEPS, 1], f32)
            nc.scalar.activation(out=actwarm, in_=lnbias, func=exp)

            # One shared grid ramp: iota_t[t, i] = i exactly in f32. Each gy
            # chunk's column offset is folded into its per-partition Square
            # bias below, so a single 1024-wide iota serves all chunks.
            iota_t = const.tile([STEPS, GY_CHUNK], f32)
            nc.gpsimd.iota(
                iota_t,
                pattern=[[1, GY_CHUNK]],
                base=0,
                channel_multiplier=0,
                allow_small_or_imprecise_dtypes=True,
            )

            # biases[:, g] = chunk_offset/RES (memset now) + negc_y (added
            # once the curve matmul lands).
            biases = const.tile([STEPS, N_GY], f32)
            for g in range(N_GY):
                nc.vector.memset(biases[:, g : g + 1], GY_OFFS[g] / RES)

            # negc[t] = (-cx[t] + rowoff, -cy[t]): one DMA + one K=5 matmul.
            comb = const.tile([5, STEPS + 2], f32)
            nc.sync.dma_start(out=comb, in_=comb_d.ap())
            negc_ps = pscurve.tile([STEPS, 2], f32)
            nc.tensor.matmul(
                out=negc_ps,
                lhsT=comb[:, :STEPS],
                rhs=comb[:, STEPS : STEPS + 2],
                start=True,
                stop=True,
            )
            negc = const.tile([STEPS, 2], f32)
            nc.vector.tensor_copy(out=negc, in_=negc_ps)
            nc.vector.tensor_scalar(
                out=biases,
                in0=biases,
                scalar1=negc[:, 1:2],
                scalar2=None,
                op0=add,
            )

            # gx = exp(-5000*(rowoff + i/8192 - cx)^2 + ln(1/100)) [STEPS,1024]
            gxs = stage.tile([STEPS, ROWS_PER_CORE], f32, tag="gys")
            nc.scalar.activation(
                out=gxs, in_=iota_t, func=square, scale=1.0 / RES, bias=negc[:, 0:1]
            )
            gx = const.tile([STEPS, ROWS_PER_CORE], g_dt)
            nc.scalar.activation(
                out=gx, in_=gxs, func=exp, scale=NEG_INV_2SIG, bias=lnbias
            )

            gy_chunks = [None] * N_GY
            copy_state = [0]

            def emit_gy_chunk(g):
                # gy chunk g = exp(-5000*((i + off_g)/8192 - cy)^2), read
                # from the shared ramp with the chunk offset folded into the
                # bias; squares alternate ACT/DVE to balance engines.
                w = GY_WIDTHS[g]
                gys = stage.tile([STEPS, w], f32, tag="gys")
                if g % 2 == 1:
                    nc.scalar.activation(
                        out=gys,
                        in_=iota_t[:, :w],
                        func=square,
                        scale=1.0 / RES,
                        bias=biases[:, g : g + 1],
                    )
                else:
                    nc.vector.tensor_scalar(
                        out=gys,
                        in0=iota_t[:, :w],
                        scalar1=inv_res,
                        scalar2=biases[:, g : g + 1],
                        op0=mult,
                        op1=add,
                    )
                    nc.vector.tensor_mul(out=gys, in0=gys, in1=gys)
                gyc = gyp.tile([STEPS, w], g_dt, tag="gyc")
                nc.scalar.activation(out=gyc, in_=gys, func=exp, scale=NEG_INV_2SIG)
                gy_chunks[g] = gyc

            def emit_col_tile(mi, g):
                # one (row-block, column-chunk) tile: 1-2 matmuls -> PSUM,
                # one PSUM->SBUF copy, one 256-512 KB store.
                row0 = mi * M_TILE
                col0 = GY_OFFS[g]
                w = GY_WIDTHS[g]
                gyc = gy_chunks[g]
                ps = psmm.tile([M_TILE, w], f32, tag="ps")
                for h in range(0, w, MM_N):
                    hw = min(MM_N, w - h)
                    nc.tensor.matmul(
                        out=ps[:, h : h + hw],
                        lhsT=gx[:, row0 : row0 + M_TILE],
                        rhs=gyc[:, h : h + hw],
                        start=True,
                        stop=True,
                    )
                ob = obuf.tile([M_TILE, w], f32, tag="ob")
                # ACT also carries Gaussian-table work; it takes ~2/5 of
                # the copies, DVE the rest.
                if copy_state[0] % 5 >= 3:
                    nc.scalar.copy(out=ob, in_=ps)
                else:
                    nc.vector.tensor_copy(out=ob, in_=ps)
                copy_state[0] += 1
                nc.sync.dma_start(
                    out=out_d.ap()[row0 : row0 + M_TILE, col0 : col0 + w],
                    in_=ob,
                )

            # --- main loop, column-major: as each gy chunk lands, all 8
            # row-blocks' matmuls for that column run and their 512 KB
            # tiles stream straight out. The DMA engines saturate right
            # after the first chunk and never wait on a row-block assembly.
            # (reps>1 wraps the steady state in a dynamic loop, for
            # benchmarking only.)
            if reps == 1:
                for g in range(N_GY):
                    emit_gy_chunk(g)
                    for mi in range(m_tiles):
                        emit_col_tile(mi, g)
            else:
                for g in range(N_GY):
                    emit_gy_chunk(g)
                with tc.For_i(0, reps, 1, hint_engines=(mybir.EngineType.PE,)):
                    for g in range(N_GY):
                        for mi in range(m_tiles):
                            emit_col_tile(mi, g)

    nc.compile()
    return nc


def _get_nc():
    if "nc" not in _CACHE:
        _CACHE["nc"] = _build_nc()
    return _CACHE["nc"]


def _host_constants():
    if "consts" not in _CACHE:
        t = np.linspace(0.0, 1.0, STEPS, dtype=np.float32).astype(np.float64)
        basis = np.stack(
            [math.comb(3, k) * (1.0 - t) ** (3 - k) * t**k for k in range(4)]
        )  # (4, STEPS) float64
        nb5 = np.concatenate(
            [-basis, np.ones((1, STEPS), np.float64)], axis=0
        ).astype(np.float32)  # (5, STEPS): [-basis; ones]
        _CACHE["consts"] = nb5
    return _CACHE["consts"]


TRACE = False
LAST_RESULT = None


def kernel(control_points: np.ndarray) -> np.ndarray:
    global LAST_RESULT
    from concourse.bass_utils import run_bass_kernel_spmd

    nc = _get_nc()
    nb5 = _host_constants()
    cp = np.ascontiguousarray(np.asarray(control_points), dtype=np.float32)

    in_maps = []
    for c in range(N_CORES):
        rowoff = np.float32(c * ROWS_PER_CORE) / np.float32(RES)
        cp5 = np.concatenate(
            [cp, np.array([[rowoff, 0.0]], np.float32)], axis=0
        )  # (5, 2)
        comb = np.concatenate([nb5, cp5], axis=1)  # (5, 102)
        in_maps.append({"curve_in": np.ascontiguousarray(comb)})

    res = run_bass_kernel_spmd(
        nc, in_maps, core_ids=list(range(N_CORES)), trace=TRACE
    )
    LAST_RESULT = res
    return np.concatenate([res.results[c]["out"] for c in range(N_CORES)], axis=0)


# revision 51
# speedup vs baseline: 1.0092x; 1.0023x over previous
"""Cubic-Bezier Gaussian rasterizer for Trainium2 (Bass/Tile), 8-core SPMD.

Math (matches reference.py):
    t = linspace(0, 1, 100);  curve = Bezier3(control_points, t)   # (2, 100)
    gx[t, i] = exp(-(curve_x[t] - i/8192)^2 / 2e-4)                # (100, 8192)
    gy[t, j] = exp(-(curve_y[t] - j/8192)^2 / 2e-4)
    out = gx^T @ gy / 100                                          # (8192, 8192)

Sharding: output rows across 8 cores. Each core computes gx for its 1024
grid-row values, the full gy, and a local (1024 x 8192) matmul. No
communication; host concatenates the row slices.

Device pipeline per core (the only DMA traffic is one 2 KB input and the
32 MB output, which is the memory-regime floor):
  PE:   negc = [neg_basis; 1]^T @ [cp; rowoff] (per-core row offset folded
        into a 5th contraction row), then 128 f32r matmuls gx^T @ gy -> PSUM
  Pool: one 1024-wide iota generates the grid ramp on-chip (exact in f32);
        each chunk's column offset is folded into its Square bias
  ACT:  Square/Exp Gaussian tables (squares alternate with DVE) + ~2/5 of
        the PSUM->SBUF copies
  DVE:  the other squares + most PSUM->SBUF copies
  DMA:  column-major 512 KB stores, issued per (row-block, column) tile so
        the DMA engines saturate right after the first gy chunk

Timing (TimelineSim cost model, cross-checked on hardware by slope-fitting
wall time over an in-kernel repetition loop): ~106 us per core end-to-end;
measured steady-state pass 102.3 us including ~4 us loop overhead.
"""

import math
import os

import numpy as np

RES = 8192
STEPS = 100
N_CORES = 8
ROWS_PER_CORE = RES // N_CORES  # 1024
NEG_INV_2SIG = -5000.0  # -1 / 0.0002
LN_INV_STEPS = float(np.log(np.float64(1.0) / STEPS))

M_TILE = 128  # output rows per PE matmul (psum partition dim)
MM_N = 512  # matmul moving free dim (one PSUM bank of f32)
PS_COLS = 1024  # psum tile free size (2 banks -> 2 matmuls per copy)
GY_CHUNK = 1024  # max gy chunk size for square/exp ops
# First chunks are narrow so the very first stores launch earlier; the
# rest use the full width. Must sum to RES.
GY_WIDTHS = [512, 512] + [1024] * 7
GY_OFFS = [sum(GY_WIDTHS[:i]) for i in range(len(GY_WIDTHS))]
N_GY = len(GY_WIDTHS)

# "f32"  : exact fp32 matmul, 4 cycles/row on the PE
# "f32r" : single-pass fp32 matmul, 1 cycle/row (relaxed multiply precision)
MM_MODE = os.environ.get("BEZ_MM_MODE", "f32r")

_CACHE = {}


def _build_nc(mm_mode=None, reps=1):
    import concourse.mybir as mybir
    import concourse.tile as tile
    from concourse import bacc

    if mm_mode is None:
        mm_mode = MM_MODE
    f32 = mybir.dt.float32
    f32r = mybir.dt.float32r
    nc = bacc.Bacc(
        "TRN2", target_bir_lowering=False, debug=False, num_devices=N_CORES
    )

    # Single tiny input: [:, :100] = [neg_basis; ones] (4+1 x 100),
    # [:, 100:102] = [control_points; [row_offset, 0]] (4+1 x 2).
    comb_d = nc.dram_tensor("curve_in", [5, STEPS + 2], f32, kind="ExternalInput")
    out_d = nc.dram_tensor("out", [ROWS_PER_CORE, RES], f32, kind="ExternalOutput")

    m_tiles = ROWS_PER_CORE // M_TILE  # 8

    exp = mybir.ActivationFunctionType.Exp
    square = mybir.ActivationFunctionType.Square
    add = mybir.AluOpType.add
    mult = mybir.AluOpType.mult

    g_dt = f32r if mm_mode == "f32r" else f32

    with tile.TileContext(nc) as tc:
        with (
            tc.tile_pool(name="const", bufs=1) as const,
            tc.tile_pool(name="gyp", bufs=N_GY) as gyp,
            tc.tile_pool(name="stage", bufs=4) as stage,
            tc.tile_pool(name="obuf", bufs=6) as obuf,
            tc.tile_pool(name="psmm", bufs=3, space="PSUM") as psmm,
            tc.tile_pool(name="pscurve", bufs=1, space="PSUM") as pscurve,
        ):
            # t=0: preload the ACT Exp/Square/Copy table via a dummy op.
            lnbias = const.tile([STEPS, 1], f32)
            nc.vector.memset(lnbias, LN_INV_STEPS)
            inv_res = const.tile([STEPS, 1], f32)
            nc.vector.memset(inv_res, 1.0 / RES)
            actwarm = const.tile([STEPS, 1], f32)
            nc.scalar.activation(out=actwarm, in_=lnbias, func=exp)

            # One shared grid ramp: iota_t[t, i] = i exactly in f32. Each gy
            # chunk's column offset is folded into its per-partition Square
            # bias below, so a single 1024-wide iota serves all chunks.
            iota_t = const.tile([STEPS, GY_CHUNK], f32)
            nc.gpsimd.iota(
                iota_t,
                pattern=[[1, GY_CHUNK]],
                base=0,
                channel_multiplier=0,
                allow_small_or_imprecise_dtypes=True,
            )

            # biases[:, g] = chunk_offset/RES (memset now) + negc_y (added
            # once the curve matmul lands).
            biases = const.tile([STEPS, N_GY], f32)
            for g in range(N_GY):
                nc.vector.memset(biases[:, g : g + 1], GY_OFFS[g] / RES)

            # negc[t] = (-cx[t] + rowoff, -cy[t]): one DMA + one K=5 matmul.
            comb = const.tile([5, STEPS + 2], f32)
            nc.sync.dma_start(out=comb, in_=comb_d.ap())
            negc_ps = pscurve.tile([STEPS, 2], f32)
            nc.tensor.matmul(
                out=negc_ps,
                lhsT=comb[:, :STEPS],
                rhs=comb[:, STEPS : STEPS + 2],
                start=True,
                stop=True,
            )
            negc = const.tile([STEPS, 2], f32)
            nc.vector.tensor_copy(out=negc, in_=negc_ps)
            nc.vector.tensor_scalar(
                out=biases,
                in0=biases,
                scalar1=negc[:, 1:2],
                scalar2=None,
                op0=add,
            )

            # gx = exp(-5000*(rowoff + i/8192 - cx)^2 + ln(1/100)), split so
            # the first output tile (row-block 0, needing only columns
            # 0..127) isn't gated on the full-width chain: gxa (128 cols,
            # ACT, ~0.6us) unblocks the first store; gxb (896 cols, DVE
            # square) computes while the first stores already stream out.
            gxa_s = stage.tile([STEPS, M_TILE], f32, tag="gys")
            nc.scalar.activation(
                out=gxa_s,
                in_=iota_t[:, :M_TILE],
                func=square,
                scale=1.0 / RES,
                bias=negc[:, 0:1],
            )
            gxa = const.tile([STEPS, M_TILE], g_dt)
            nc.scalar.activation(
                out=gxa, in_=gxa_s, func=exp, scale=NEG_INV_2SIG, bias=lnbias
            )

            gxb = None  # emitted after the first column tile, see below

            def emit_gxb():
                gxb_s = stage.tile([STEPS, ROWS_PER_CORE - M_TILE], f32, tag="gys")
                nc.vector.tensor_scalar(
                    out=gxb_s,
                    in0=iota_t[:, M_TILE:ROWS_PER_CORE],
                    scalar1=inv_res,
                    scalar2=negc[:, 0:1],
                    op0=mult,
                    op1=add,
                )
                nc.vector.tensor_mul(out=gxb_s, in0=gxb_s, in1=gxb_s)
                t = const.tile([STEPS, ROWS_PER_CORE - M_TILE], g_dt)
                nc.scalar.activation(
                    out=t, in_=gxb_s, func=exp, scale=NEG_INV_2SIG, bias=lnbias
                )
                return t

            gy_chunks = [None] * N_GY
            copy_state = [0]

            def emit_gy_chunk(g):
                # gy chunk g = exp(-5000*((i + off_g)/8192 - cy)^2), read
                # from the shared ramp with the chunk offset folded into the
                # bias; squares alternate ACT/DVE to balance engines.
                w = GY_WIDTHS[g]
                gys = stage.tile([STEPS, w], f32, tag="gys")
                if g % 2 == 1:
                    nc.scalar.activation(
                        out=gys,
                        in_=iota_t[:, :w],
                        func=square,
                        scale=1.0 / RES,
                        bias=biases[:, g : g + 1],
                    )
                else:
                    nc.vector.tensor_scalar(
                        out=gys,
                        in0=iota_t[:, :w],
                        scalar1=inv_res,
                        scalar2=biases[:, g : g + 1],
                        op0=mult,
                        op1=add,
                    )
                    nc.vector.tensor_mul(out=gys, in0=gys, in1=gys)
                gyc = gyp.tile([STEPS, w], g_dt, tag="gyc")
                nc.scalar.activation(out=gyc, in_=gys, func=exp, scale=NEG_INV_2SIG)
                gy_chunks[g] = gyc

            def emit_col_tile(mi, g):
                # one (row-block, column-chunk) tile: 1-2 matmuls -> PSUM,
                # one PSUM->SBUF copy, one 256-512 KB store.
                row0 = mi * M_TILE
                col0 = GY_OFFS[g]
                w = GY_WIDTHS[g]
                gyc = gy_chunks[g]
                lhsT = (
                    gxa if mi == 0 else gxb[:, row0 - M_TILE : row0]
                )
                ps = psmm.tile([M_TILE, w], f32, tag="ps")
                for h in range(0, w, MM_N):
                    hw = min(MM_N, w - h)
                    nc.tensor.matmul(
                        out=ps[:, h : h + hw],
                        lhsT=lhsT,
                        rhs=gyc[:, h : h + hw],
                        start=True,
                        stop=True,
                    )
                ob = obuf.tile([M_TILE, w], f32, tag="ob")
                # ACT also carries Gaussian-table work; it takes ~2/5 of
                # the copies, DVE the rest.
                if copy_state[0] % 5 >= 3:
                    nc.scalar.copy(out=ob, in_=ps)
                else:
                    nc.vector.tensor_copy(out=ob, in_=ps)
                copy_state[0] += 1
                nc.sync.dma_start(
                    out=out_d.ap()[row0 : row0 + M_TILE, col0 : col0 + w],
                    in_=ob,
                )

            # --- main loop, column-major: as each gy chunk lands, all 8
            # row-blocks' matmuls for that column run and their 512 KB
            # tiles stream straight out. The DMA engines saturate right
            # after the first chunk and never wait on a row-block assembly.
            # (reps>1 wraps the steady state in a dynamic loop, for
            # benchmarking only.)
            if reps == 1:
                emit_gy_chunk(0)
                emit_col_tile(0, 0)  # first store: gxa + chunk 0 only
                # overlaps the first store; deprioritized so the scheduler
                # doesn't slot its DVE square into the chunk-0 chain
                with tc.high_priority(-12):
                    gxb = emit_gxb()
                for mi in range(1, m_tiles):
                    emit_col_tile(mi, 0)
                for g in range(1, N_GY):
                    emit_gy_chunk(g)
                    for mi in range(m_tiles):
                        emit_col_tile(mi, g)
            else:
                gxb = emit_gxb()
                for g in range(N_GY):
                    emit_gy_chunk(g)
                with tc.For_i(0, reps, 1, hint_engines=(mybir.EngineType.PE,)):
                    for g in range(N_GY):
                        for mi in range(m_tiles):
                            emit_col_tile(mi, g)

    nc.compile()
    return nc


def _get_nc():
    if "nc" not in _CACHE:
        _CACHE["nc"] = _build_nc()
    return _CACHE["nc"]


def _host_constants():
    if "consts" not in _CACHE:
        t = np.linspace(0.0, 1.0, STEPS, dtype=np.float32).astype(np.float64)
        basis = np.stack(
            [math.comb(3, k) * (1.0 - t) ** (3 - k) * t**k for k in range(4)]
        )  # (4, STEPS) float64
        nb5 = np.concatenate(
            [-basis, np.ones((1, STEPS), np.float64)], axis=0
        ).astype(np.float32)  # (5, STEPS): [-basis; ones]
        _CACHE["consts"] = nb5
    return _CACHE["consts"]


TRACE = False
LAST_RESULT = None


def kernel(control_points: np.ndarray) -> np.ndarray:
    global LAST_RESULT
    from concourse.bass_utils import run_bass_kernel_spmd

    nc = _get_nc()
    nb5 = _host_constants()
    cp = np.ascontiguousarray(np.asarray(control_points), dtype=np.float32)

    in_maps = []
    for c in range(N_CORES):
        rowoff = np.float32(c * ROWS_PER_CORE) / np.float32(RES)
        cp5 = np.concatenate(
            [cp, np.array([[rowoff, 0.0]], np.float32)], axis=0
        )  # (5, 2)
        comb = np.concatenate([nb5, cp5], axis=1)  # (5, 102)
        in_maps.append({"curve_in": np.ascontiguousarray(comb)})

    res = run_bass_kernel_spmd(
        nc, in_maps, core_ids=list(range(N_CORES)), trace=TRACE
    )
    LAST_RESULT = res
    return np.concatenate([res.results[c]["out"] for c in range(N_CORES)], axis=0)


# revision 54
# speedup vs baseline: 1.0155x; 1.0062x over previous
"""Cubic-Bezier Gaussian rasterizer for Trainium2 (Bass/Tile), 8-core SPMD.

Math (matches reference.py):
    t = linspace(0, 1, 100);  curve = Bezier3(control_points, t)   # (2, 100)
    gx[t, i] = exp(-(curve_x[t] - i/8192)^2 / 2e-4)                # (100, 8192)
    gy[t, j] = exp(-(curve_y[t] - j/8192)^2 / 2e-4)
    out = gx^T @ gy / 100                                          # (8192, 8192)

Sharding: output rows across 8 cores. Each core computes gx for its 1024
grid-row values, the full gy, and a local (1024 x 8192) matmul. No
communication; host concatenates the row slices.

Device pipeline per core (the only DMA traffic is one 2 KB input and the
32 MB output, which is the memory-regime floor):
  PE:   negc = [neg_basis; 1]^T @ [cp; rowoff] (per-core row offset folded
        into a 5th contraction row), then 128 f32r matmuls gx^T @ gy -> PSUM
  Pool: one 1024-wide iota generates the grid ramp on-chip (exact in f32);
        each chunk's column offset is folded into its Square bias
  ACT:  Square/Exp Gaussian tables (squares alternate with DVE) + ~2/5 of
        the PSUM->SBUF copies
  DVE:  the other squares + most PSUM->SBUF copies
  DMA:  column-major 512 KB stores, issued per (row-block, column) tile so
        the DMA engines saturate right after the first gy chunk

Timing (TimelineSim cost model, cross-checked on hardware by slope-fitting
wall time over an in-kernel repetition loop): ~104.6 us per core
end-to-end (8.5 us pipeline fill + 94.3 us saturated output stream +
1.6 us drain); measured steady-state pass 102-107 us including ~4 us
loop overhead. The stream runs at ~343 GB/s effective per core with all
8 cores writing concurrently, ~95% of the per-NeuronCore HBM bound.
"""

import math
import os

import numpy as np

RES = 8192
STEPS = 100
N_CORES = 8
ROWS_PER_CORE = RES // N_CORES  # 1024
NEG_INV_2SIG = -5000.0  # -1 / 0.0002
LN_INV_STEPS = float(np.log(np.float64(1.0) / STEPS))

M_TILE = 128  # output rows per PE matmul (psum partition dim)
MM_N = 512  # matmul moving free dim (one PSUM bank of f32)
PS_COLS = 1024  # psum tile free size (2 banks -> 2 matmuls per copy)
GY_CHUNK = 1024  # max gy chunk size for square/exp ops
# First chunks are narrow so the very first stores launch earlier; the
# rest use the full width. Must sum to RES.
GY_WIDTHS = [512, 512] + [1024] * 7
GY_OFFS = [sum(GY_WIDTHS[:i]) for i in range(len(GY_WIDTHS))]
N_GY = len(GY_WIDTHS)

# "f32"  : exact fp32 matmul, 4 cycles/row on the PE
# "f32r" : single-pass fp32 matmul, 1 cycle/row (relaxed multiply precision)
MM_MODE = os.environ.get("BEZ_MM_MODE", "f32r")

_CACHE = {}


def _build_nc(mm_mode=None, reps=1):
    import concourse.mybir as mybir
    import concourse.tile as tile
    from concourse import bacc

    if mm_mode is None:
        mm_mode = MM_MODE
    f32 = mybir.dt.float32
    f32r = mybir.dt.float32r
    nc = bacc.Bacc(
        "TRN2", target_bir_lowering=False, debug=False, num_devices=N_CORES
    )

    # Single tiny input: [:, :100] = [neg_basis; ones] (4+1 x 100),
    # [:, 100:102] = [control_points; [row_offset, 0]] (4+1 x 2).
    comb_d = nc.dram_tensor("curve_in", [5, STEPS + 2], f32, kind="ExternalInput")
    out_d = nc.dram_tensor("out", [ROWS_PER_CORE, RES], f32, kind="ExternalOutput")

    m_tiles = ROWS_PER_CORE // M_TILE  # 8

    exp = mybir.ActivationFunctionType.Exp
    square = mybir.ActivationFunctionType.Square
    add = mybir.AluOpType.add
    mult = mybir.AluOpType.mult

    g_dt = f32r if mm_mode == "f32r" else f32

    with tile.TileContext(nc) as tc:
        with (
            tc.tile_pool(name="const", bufs=1) as const,
            tc.tile_pool(name="gyp", bufs=N_GY) as gyp,
            tc.tile_pool(name="stage", bufs=4) as stage,
            tc.tile_pool(name="obuf", bufs=8) as obuf,
            tc.tile_pool(name="psmm", bufs=3, space="PSUM") as psmm,
            tc.tile_pool(name="pscurve", bufs=1, space="PSUM") as pscurve,
        ):
            # t=0: preload the ACT Exp/Square/Copy table via a dummy op.
            lnbias = const.tile([STEPS, 1], f32)
            nc.vector.memset(lnbias, LN_INV_STEPS)
            inv_res = const.tile([STEPS, 1], f32)
            nc.vector.memset(inv_res, 1.0 / RES)
            actwarm = const.tile([STEPS, 1], f32)
            nc.scalar.activation(out=actwarm, in_=lnbias, func=exp)

            # One shared grid ramp: iota_t[t, i] = i exactly in f32. Each gy
            # chunk's column offset is folded into its per-partition Square
            # bias below, so a single 1024-wide iota serves all chunks.
            iota_t = const.tile([STEPS, GY_CHUNK], f32)
            nc.gpsimd.iota(
                iota_t,
                pattern=[[1, GY_CHUNK]],
                base=0,
                channel_multiplier=0,
                allow_small_or_imprecise_dtypes=True,
            )

            # biases[:, g] = chunk_offset/RES (memset now) + negc_y (added
            # once the curve matmul lands).
            biases = const.tile([STEPS, N_GY], f32)
            for g in range(N_GY):
                nc.vector.memset(biases[:, g : g + 1], GY_OFFS[g] / RES)

            # negc[t] = (-cx[t] + rowoff, -cy[t]): one DMA + one K=5 matmul.
            comb = const.tile([5, STEPS + 2], f32)
            nc.sync.dma_start(out=comb, in_=comb_d.ap())
            negc_ps = pscurve.tile([STEPS, 2], f32)
            nc.tensor.matmul(
                out=negc_ps,
                lhsT=comb[:, :STEPS],
                rhs=comb[:, STEPS : STEPS + 2],
                start=True,
                stop=True,
            )
            negc = const.tile([STEPS, 2], f32)
            nc.vector.tensor_copy(out=negc, in_=negc_ps)
            nc.vector.tensor_scalar(
                out=biases,
                in0=biases,
                scalar1=negc[:, 1:2],
                scalar2=None,
                op0=add,
            )

            # gx = exp(-5000*(rowoff + i/8192 - cx)^2 + ln(1/100)), split so
            # the first output tile (row-block 0, needing only columns
            # 0..127) isn't gated on the full-width chain: gxa (128 cols,
            # ACT, ~0.6us) unblocks the first store; gxb (896 cols, DVE
            # square) computes while the first stores already stream out.
            gxa_s = stage.tile([STEPS, M_TILE], f32, tag="gys")
            nc.scalar.activation(
                out=gxa_s,
                in_=iota_t[:, :M_TILE],
                func=square,
                scale=1.0 / RES,
                bias=negc[:, 0:1],
            )
            gxa = const.tile([STEPS, M_TILE], g_dt)
            nc.scalar.activation(
                out=gxa, in_=gxa_s, func=exp, scale=NEG_INV_2SIG, bias=lnbias
            )

            gxb = None  # emitted after the first column tile, see below

            def emit_gxb():
                gxb_s = stage.tile([STEPS, ROWS_PER_CORE - M_TILE], f32, tag="gys")
                nc.vector.tensor_scalar(
                    out=gxb_s,
                    in0=iota_t[:, M_TILE:ROWS_PER_CORE],
                    scalar1=inv_res,
                    scalar2=negc[:, 0:1],
                    op0=mult,
                    op1=add,
                )
                nc.vector.tensor_mul(out=gxb_s, in0=gxb_s, in1=gxb_s)
                t = const.tile([STEPS, ROWS_PER_CORE - M_TILE], g_dt)
                nc.scalar.activation(
                    out=t, in_=gxb_s, func=exp, scale=NEG_INV_2SIG, bias=lnbias
                )
                return t

            gy_chunks = [None] * N_GY
            copy_state = [0]

            def emit_gy_chunk(g):
                # gy chunk g = exp(-5000*((i + off_g)/8192 - cy)^2), read
                # from the shared ramp with the chunk offset folded into the
                # bias; squares alternate ACT/DVE to balance engines.
                w = GY_WIDTHS[g]
                gys = stage.tile([STEPS, w], f32, tag="gys")
                if g % 2 == 1:
                    nc.scalar.activation(
                        out=gys,
                        in_=iota_t[:, :w],
                        func=square,
                        scale=1.0 / RES,
                        bias=biases[:, g : g + 1],
                    )
                else:
                    nc.vector.tensor_scalar(
                        out=gys,
                        in0=iota_t[:, :w],
                        scalar1=inv_res,
                        scalar2=biases[:, g : g + 1],
                        op0=mult,
                        op1=add,
                    )
                    nc.vector.tensor_mul(out=gys, in0=gys, in1=gys)
                gyc = gyp.tile([STEPS, w], g_dt, tag="gyc")
                nc.scalar.activation(out=gyc, in_=gys, func=exp, scale=NEG_INV_2SIG)
                gy_chunks[g] = gyc

            def emit_col_tile(mi, g):
                # one (row-block, column-chunk) tile: 1-2 matmuls -> PSUM,
                # one PSUM->SBUF copy, one 256-512 KB store.
                row0 = mi * M_TILE
                col0 = GY_OFFS[g]
                w = GY_WIDTHS[g]
                gyc = gy_chunks[g]
                lhsT = (
                    gxa if mi == 0 else gxb[:, row0 - M_TILE : row0]
                )
                ps = psmm.tile([M_TILE, w], f32, tag="ps")
                for h in range(0, w, MM_N):
                    hw = min(MM_N, w - h)
                    nc.tensor.matmul(
                        out=ps[:, h : h + hw],
                        lhsT=lhsT,
                        rhs=gyc[:, h : h + hw],
                        start=True,
                        stop=True,
                    )
                ob = obuf.tile([M_TILE, w], f32, tag="ob")
                # PSUM->SBUF copies alternate ACT/DVE evenly
                if copy_state[0] % 2 == 1:
                    nc.scalar.copy(out=ob, in_=ps)
                else:
                    nc.vector.tensor_copy(out=ob, in_=ps)
                copy_state[0] += 1
                nc.sync.dma_start(
                    out=out_d.ap()[row0 : row0 + M_TILE, col0 : col0 + w],
                    in_=ob,
                )

            # --- main loop, column-major: as each gy chunk lands, all 8
            # row-blocks' matmuls for that column run and their 512 KB
            # tiles stream straight out. The DMA engines saturate right
            # after the first chunk and never wait on a row-block assembly.
            # (reps>1 wraps the steady state in a dynamic loop, for
            # benchmarking only.)
            if reps == 1:
                emit_gy_chunk(0)
                emit_col_tile(0, 0)  # first store: gxa + chunk 0 only
                # overlaps the first store; deprioritized so the scheduler
                # doesn't slot its DVE square into the chunk-0 chain
                with tc.high_priority(-12):
                    gxb = emit_gxb()
                for mi in range(1, m_tiles):
                    emit_col_tile(mi, 0)
                for g in range(1, N_GY):
                    emit_gy_chunk(g)
                    for mi in range(m_tiles):
                        emit_col_tile(mi, g)
            else:
                gxb = emit_gxb()
                for g in range(N_GY):
                    emit_gy_chunk(g)
                with tc.For_i(0, reps, 1, hint_engines=(mybir.EngineType.PE,)):
                    for g in range(N_GY):
                        for mi in range(m_tiles):
                            emit_col_tile(mi, g)

    nc.compile()
    return nc


def _get_nc():
    if "nc" not in _CACHE:
        _CACHE["nc"] = _build_nc()
    return _CACHE["nc"]


def _host_constants():
    if "consts" not in _CACHE:
        t = np.linspace(0.0, 1.0, STEPS, dtype=np.float32).astype(np.float64)
        basis = np.stack(
            [math.comb(3, k) * (1.0 - t) ** (3 - k) * t**k for k in range(4)]
        )  # (4, STEPS) float64
        nb5 = np.concatenate(
            [-basis, np.ones((1, STEPS), np.float64)], axis=0
        ).astype(np.float32)  # (5, STEPS): [-basis; ones]
        _CACHE["consts"] = nb5
    return _CACHE["consts"]


TRACE = False
LAST_RESULT = None


def kernel(control_points: np.ndarray) -> np.ndarray:
    global LAST_RESULT
    from concourse.bass_utils import run_bass_kernel_spmd

    nc = _get_nc()
    nb5 = _host_constants()
    cp = np.ascontiguousarray(np.asarray(control_points), dtype=np.float32)

    in_maps = []
    for c in range(N_CORES):
        rowoff = np.float32(c * ROWS_PER_CORE) / np.float32(RES)
        cp5 = np.concatenate(
            [cp, np.array([[rowoff, 0.0]], np.float32)], axis=0
        )  # (5, 2)
        comb = np.concatenate([nb5, cp5], axis=1)  # (5, 102)
        in_maps.append({"curve_in": np.ascontiguousarray(comb)})

    res = run_bass_kernel_spmd(
        nc, in_maps, core_ids=list(range(N_CORES)), trace=TRACE
    )
    LAST_RESULT = res
    return np.concatenate([res.results[c]["out"] for c in range(N_CORES)], axis=0)
